# revision 46
# baseline (speedup 1.0000x reference)
"""Causal self-attention Trainium2 kernel (B=1, S=4096, E=1024, H=16, D=64).

Sharding: tensor-parallel over heads - 2 heads per core (8 cores).
Each core computes Q/K/V for its 2 heads, causal attention, and a partial
o_proj over its 128 output-feature slice; the host sums the 8 partials.

Key structure (per core), v2:
  * Q/K/V projections run as fp8e4 DoubleRow matmuls (x and 64*W shipped in
    fp8; contraction 256 per matmul, 0.5 cyc/col) -> bf16 K/Q tiles carry a
    x64 scale that is folded into the exp scale (logits x4096); V is scaled
    back by 1/64 in its PSUM->fp8 cast.
  * q-tile 0 (rows 0-511, low-context -> accuracy-critical) instead uses a
    bf16-exact path: bf16 x / bf16 W projections, exact exp, bf16 V.
  * softmax exp+fp8-quantize is spread across THREE engines: ACT does exact
    exp->fp8e4 for its share; DVE and GPSIMD exploit that fp8e4's bit
    pattern is log-linear, so bits(fp8(e^(s*lg+b))) ~= clamp(a*lg+b', 0, *):
    a 2-instr affine (tensor_scalar mult/add -> bf16, then max(.,0) -> uint8
    written through a bitcast view of the fp8 tile). A uniform bit offset is
    a uniform weight scale and cancels in the softmax normalization.
  * logits computed transposed, both heads packed into PE row-groups
    (rows 0-63 / 64-127) writing separate PSUM banks.
  * causal masking via GPSIMD affine_select on diagonal-band blocks.
  * PV uses fp8e4 DoubleRow matmuls (kv-pair contraction 256) with an
    appended ones column accumulating the softmax denominator in row 64.
  * normalize via reciprocal_approx_fast + stream_shuffle broadcast.
  * per-q-tile projections and o_proj matmuls interleave as PE fillers.
"""

import math
import sys
from collections import deque

import numpy as np

for _p in ("/opt/trn_rl_repo", "/opt/trn_rl_repo/concourse"):
    if _p not in sys.path:
        sys.path.insert(0, _p)

import ml_dtypes

BF16 = ml_dtypes.bfloat16
F8E4 = ml_dtypes.float8_e4m3

S = 4096
E = 1024
H = 16
D = 64
NCORES = 8
QT = 512  # query tile (free dim of logits matmuls)
NQ = S // QT  # 8
KB = 128  # kv block (partition dim of logits tiles)
SCALE = 1.0 / math.sqrt(D)
LOG2E = math.log2(math.e)

EXPB = -1.5  # exp(scale*logit + EXPB); uniform factor cancels in softmax
B_AFF = 8.0 * (7.0 + LOG2E * EXPB)  # fp8e4 bits bias (38.688)
A_RAW = 8.0 * LOG2E * SCALE  # bits per raw-logit unit (1.4427)
A_FP8 = A_RAW / 4096.0  # fp8-proj logits carry x4096
WSC = 64.0  # host weight pre-scale for fp8

# K/Q casts pre-scale by M_KQ so the fp8-path logits arrive in fp8e4 "bits"
# scale: bits = LG_scaled + B_AFF. The DVE exp is then ONE tensor_scalar
# (add B, max 0) -> uint8; ACT uses exact exp (scale ln2/8 recovers nats).
M_KQ = math.sqrt(A_RAW) / WSC
# exp engine shares for non-diagonal tiles (diag + qi0 go to ACT).
W_ACT, W_DVE = 0.55, 0.45

_CACHE = {}


def _build_nc():
    import concourse.tile as tile
    from concourse import bacc, mybir

    dt = mybir.dt
    f32 = dt.float32
    bf16 = dt.bfloat16
    fp8 = dt.float8e4
    u8 = dt.uint8
    Exp = mybir.ActivationFunctionType.Exp
    Copy = mybir.ActivationFunctionType.Copy
    DoubleRow = mybir.MatmulPerfMode.DoubleRow
    Alu = mybir.AluOpType

    nc = bacc.Bacc("TRN2", target_bir_lowering=False, debug=False, num_devices=NCORES)

    xT8_d = nc.dram_tensor("xT8", [128, 8 * S], fp8, kind="ExternalInput")
    xTb_d = nc.dram_tensor("xTb", [128, 8 * QT], bf16, kind="ExternalInput")
    wq8_d = nc.dram_tensor("wq8", [128, 1024], fp8, kind="ExternalInput")
    wk8_d = nc.dram_tensor("wk8", [128, 1024], fp8, kind="ExternalInput")
    wv8_d = nc.dram_tensor("wv8", [128, 1024], fp8, kind="ExternalInput")
    wqb_d = nc.dram_tensor("wqb", [128, 1024], bf16, kind="ExternalInput")
    wkb_d = nc.dram_tensor("wkb", [128, 1024], bf16, kind="ExternalInput")
    wvb_d = nc.dram_tensor("wvb", [128, 1024], bf16, kind="ExternalInput")
    wo_d = nc.dram_tensor("wo", [128, 1024], bf16, kind="ExternalInput")
    out_d = nc.dram_tensor("out", [S, E], bf16, kind="ExternalOutput")

    with tile.TileContext(nc) as tc:
        from contextlib import ExitStack

        with ExitStack() as ctx:
            sb = ctx.enter_context(tc.tile_pool(name="sb", bufs=1))
            lgp = ctx.enter_context(tc.tile_pool(name="lgp", bufs=2, space="PSUM"))
            ps = ctx.enter_context(tc.tile_pool(name="ps", bufs=2, space="PSUM"))
            pvp = ctx.enter_context(tc.tile_pool(name="pvp", bufs=2, space="PSUM"))
            expp = ctx.enter_context(tc.tile_pool(name="expp", bufs=8))
            tmpp = ctx.enter_context(tc.tile_pool(name="tmpp", bufs=4))
            normp = ctx.enter_context(tc.tile_pool(name="normp", bufs=3))
            ostp = ctx.enter_context(tc.tile_pool(name="ostp", bufs=4))

            # ---- persistent SBUF tensors + input DMA ----
            wq8_sb = sb.tile([128, 1024], fp8, name="wq8_sb", tag="wq8_sb")
            wk8_sb = sb.tile([128, 1024], fp8, name="wk8_sb", tag="wk8_sb")
            wv8_sb = sb.tile([128, 1024], fp8, name="wv8_sb", tag="wv8_sb")
            wqb_sb = sb.tile([128, 1024], bf16, name="wqb_sb", tag="wqb_sb")
            wkb_sb = sb.tile([128, 1024], bf16, name="wkb_sb", tag="wkb_sb")
            wvb_sb = sb.tile([128, 1024], bf16, name="wvb_sb", tag="wvb_sb")
            wo_sb = sb.tile([128, 1024], bf16, name="wo_sb", tag="wo_sb")
            nc.sync.dma_start(wkb_sb[:], wkb_d[:])
            nc.sync.dma_start(wqb_sb[:], wqb_d[:])
            nc.sync.dma_start(wvb_sb[:], wvb_d[:])
            nc.sync.dma_start(wk8_sb[:], wk8_d[:])
            nc.sync.dma_start(wq8_sb[:], wq8_d[:])
            nc.sync.dma_start(wv8_sb[:], wv8_d[:])
            nc.sync.dma_start(wo_sb[:], wo_d[:])

            xtb = sb.tile([128, 8 * QT], bf16, name="xtb", tag="xtb")
            nc.sync.dma_start(xtb[:], xTb_d[:])
            xt8 = sb.tile([128, 8 * S], fp8, name="xt8", tag="xt8")
            for ec in range(8):
                nc.sync.dma_start(
                    xt8[:, ec * S : ec * S + 2 * QT],
                    xT8_d[:, ec * S : ec * S + 2 * QT],
                )
            for ec in range(8):
                nc.sync.dma_start(
                    xt8[:, ec * S + 2 * QT : (ec + 1) * S],
                    xT8_d[:, ec * S + 2 * QT : (ec + 1) * S],
                )
            x8v = xt8[:].rearrange("p (c s) -> p c s", c=8)
            xbv = xtb[:].rearrange("p (c s) -> p c s", c=8)

            kts = [sb.tile([128, QT], bf16, name=f"kt{i}", tag=f"kt{i}") for i in range(NQ)]
            qts = [sb.tile([128, QT], bf16, name=f"qt{i}", tag=f"qt{i}") for i in range(NQ)]
            kt0x = sb.tile([128, QT], bf16, name="kt0x", tag="kt0x")
            qt0x = sb.tile([128, QT], bf16, name="qt0x", tag="qt0x")
            aots = [sb.tile([128, QT], bf16, name=f"ao{i}", tag=f"ao{i}") for i in range(NQ)]
            # V for DoubleRow PV: one fp8 tile per kv-block PAIR, layout
            # [128(s within block), pair-slot(2) x 160]: head A V at d 0-63 +
            # ones col 64; head B V at 80-143 + ones col 144.
            v8s = []
            for i in range(16):
                v = sb.tile([128, 320], fp8, name=f"v{i}", tag=f"v{i}")
                vv = v[:].rearrange("p (t d) -> p t d", t=2)
                nc.vector.memset(vv[:, :, 64:65], 1.0)
                nc.vector.memset(vv[:, :, 144:145], 1.0)
                v8s.append(v)
            # bf16 V for q-tile 0 (exact path)
            vb16 = []
            for i in range(4):
                v = sb.tile([128, 130], bf16, name=f"vb{i}", tag=f"vb{i}")
                nc.vector.memset(v[:, 64:65], 1.0)
                nc.vector.memset(v[:, 129:130], 1.0)
                vb16.append(v)

            bcseed = sb.tile([64, QT], f32, name="bcseed", tag="bcseed")
            nc.vector.memset(bcseed[:], 0.0)
            ebias = sb.tile([128, 1], f32, name="ebias", tag="ebias")
            nc.vector.memset(ebias[:], EXPB)

            # ---- filler-unit constructors ----
            def kq8_units(dst, w8, st):
                # fp8 DoubleRow K/Q projection of q-tile st -> dst bf16 (x64)
                cols = slice(st * QT, (st + 1) * QT)
                w8v = w8[:].rearrange("p (c d) -> p c d", c=8)
                state = {}

                def mm(i):
                    def f():
                        if i == 0:
                            state["t"] = ps.tile([128, QT], f32, name="ps_kq", tag="ps")
                        nc.tensor.matmul(
                            state["t"][:],
                            lhsT=w8v[:, 2 * i : 2 * i + 2, :],
                            rhs=x8v[:, 2 * i : 2 * i + 2, cols],
                            start=(i == 0),
                            stop=(i == 3),
                            perf_mode=DoubleRow,
                        )

                    return f

                def cast():
                    nc.scalar.activation(
                        dst[:], state["t"][:], Copy, scale=M_KQ
                    )

                return [mm(i) for i in range(4)] + [cast]

            def v8_units(kb):
                # fp8 DoubleRow V projection of kv block kb -> v8s (x1/64)
                scols = slice(kb * 128, (kb + 1) * 128)
                w8v = wv8_sb[:].rearrange("p (c d) -> p c d", c=8)
                state = {}

                def mm(i):
                    def f():
                        if i == 0:
                            state["t"] = ps.tile([128, 128], f32, name="ps_v", tag="ps")
                        nc.tensor.matmul(
                            state["t"][:],
                            lhsT=x8v[:, 2 * i : 2 * i + 2, scols],
                            rhs=w8v[:, 2 * i : 2 * i + 2, :],
                            start=(i == 0),
                            stop=(i == 3),
                            perf_mode=DoubleRow,
                        )

                    return f

                def cast():
                    # both head slices in one strided instr: d 0:64 -> slot
                    # cols 0:64, d 64:128 -> cols 80:144 (stride-80 pair dim)
                    r = kb % 2
                    src = state["t"][:].rearrange("p (h d) -> p h d", h=2)
                    dst = v8s[kb // 2][:, r * 160 : (r + 1) * 160].rearrange(
                        "p (h d) -> p h d", h=2
                    )[:, :, 0:64]
                    nc.scalar.activation(dst, src, Copy, scale=1.0 / WSC)

                return [mm(i) for i in range(4)] + [cast]

            def kqx_units(dst, wb):
                # bf16-exact K/Q projection of q-tile 0
                state = {}

                def mm(ec):
                    def f():
                        if ec == 0:
                            state["t"] = ps.tile([128, QT], f32, name="ps_kqx", tag="ps")
                        nc.tensor.matmul(
                            state["t"][:],
                            lhsT=wb[:, ec * 128 : (ec + 1) * 128],
                            rhs=xbv[:, ec, :],
                            start=(ec == 0),
                            stop=(ec == 7),
                        )

                    return f

                def cast():
                    nc.vector.tensor_copy(dst[:], state["t"][:])

                return [mm(ec) for ec in range(8)] + [cast]

            def vx_units(kb):
                # bf16-exact V block kb (0..3) -> vb16
                state = {}

                def mm(ec):
                    def f():
                        if ec == 0:
                            state["t"] = ps.tile([128, 128], f32, name="ps_vx", tag="ps")
                        nc.tensor.matmul(
                            state["t"][:],
                            lhsT=xbv[:, ec, kb * 128 : (kb + 1) * 128],
                            rhs=wvb_sb[:, ec * 128 : (ec + 1) * 128],
                            start=(ec == 0),
                            stop=(ec == 7),
                        )

                    return f

                def cast():
                    nc.vector.tensor_copy(vb16[kb][:, 0:64], state["t"][:, 0:64])
                    nc.vector.tensor_copy(vb16[kb][:, 65:129], state["t"][:, 64:128])

                return [mm(ec) for ec in range(8)] + [cast]

            def oproj_units(qj, sbis=range(4), tail=False):
                units = []
                for sbi in sbis:
                    for half in range(2):

                        def f(sbi=sbi, half=half):
                            srow = qj * QT + sbi * 128
                            po = ps.tile([128, 512], f32, name="po", tag="ps")
                            nc.tensor.matmul(
                                po[:],
                                lhsT=aots[qj][:, sbi * 128 : (sbi + 1) * 128],
                                rhs=wo_sb[:, half * 512 : (half + 1) * 512],
                                start=True,
                                stop=True,
                            )
                            ost = ostp.tile([128, 512], bf16, name="ost", tag="ost")
                            if (tail and half == 0) or (sbi + half) % 2 == 0:
                                nc.scalar.copy(ost[:], po[:])
                            else:
                                nc.vector.tensor_copy(ost[:], po[:])
                            nc.sync.dma_start(
                                out_d[srow : srow + 128, half * 512 : (half + 1) * 512],
                                ost[:],
                            )

                        units.append(f)
                return units

            def proj8_units(qi2):
                u = []
                u += kq8_units(kts[qi2], wk8_sb, qi2)
                u += kq8_units(qts[qi2], wq8_sb, qi2)
                for kb in range(4 * qi2, 4 * qi2 + 4):
                    u += v8_units(kb)
                return u

            def norm_emit(pvA, pvB, qj, c0, c1, tail=False):
                for pv, r0 in ((pvA, 0), (pvB, 64)):
                    den_sb = normp.tile([1, QT], f32, name="den_sb", tag="den")
                    if tail:
                        nc.scalar.copy(den_sb[:, c0:c1], pv[64:65, c0:c1])
                    else:
                        nc.vector.tensor_copy(den_sb[:, c0:c1], pv[64:65, c0:c1])
                    nc.vector.reciprocal_approx_fast(
                        bcseed[0:1, c0:c1], den_sb[:, c0:c1]
                    )
                    nc.vector.tensor_copy(bcseed[32:33, c0:c1], bcseed[0:1, c0:c1])
                    bcast = normp.tile([64, QT], f32, name="bcast", tag="bcast")
                    nc.vector.stream_shuffle(
                        bcast[:, c0:c1], bcseed[:, c0:c1], [0] * 32
                    )
                    nc.vector.tensor_mul(
                        aots[qj][r0 : r0 + 64, c0:c1], pv[0:64, c0:c1],
                        bcast[:, c0:c1],
                    )

            # weighted round-robin chooser for non-diag exp engine
            eng_credit = {"act": 0.0, "dve": 0.0}
            eng_w = {"act": W_ACT, "dve": W_DVE}

            def pick_engine():
                for k in eng_credit:
                    eng_credit[k] += eng_w[k]
                best = max(eng_credit, key=lambda k: eng_credit[k])
                eng_credit[best] -= 1.0
                return best

            # ---- prologue: q-tile 0 exact projections + fp8 tile-0 ----
            for f in kqx_units(kt0x, wkb_sb):
                f()
            for f in kqx_units(qt0x, wqb_sb):
                f()
            for kb in range(4):
                for f in vx_units(kb):
                    f()
            for f in proj8_units(0):
                f()

            # ---- main loop over q-tiles ----
            for qi in range(NQ):
                fillers = deque()
                if qi + 1 < NQ:
                    fillers.extend(proj8_units(qi + 1))
                if qi >= 1:
                    fillers.extend(oproj_units(qi - 1))

                n_kb = 4 * (qi + 1)
                pvA = pvp.tile([65, QT], f32, name="pvA", tag="pv")
                pvB = pvp.tile([65, QT], f32, name="pvB", tag="pv")
                if qi == 0:
                    # bf16-exact path for rows 0-511
                    for kb in range(4):
                        kvs = slice(kb * KB, (kb + 1) * KB)
                        off = kb
                        qlo = off * KB
                        nq = QT - qlo
                        lg = lgp.tile([128, 2 * QT], f32, name="lg", tag="lg")
                        nc.tensor.matmul(
                            lg[:, qlo:QT], lhsT=kt0x[0:64, kvs],
                            rhs=qt0x[0:64, qlo:QT], start=True, stop=True,
                        )
                        nc.tensor.matmul(
                            lg[:, QT + qlo : 2 * QT], lhsT=kt0x[64:128, kvs],
                            rhs=qt0x[64:128, qlo:QT], start=True, stop=True,
                        )
                        exb = expp.tile([128, 2 * QT], bf16, name="exb", tag="exp")
                        lg_v = lg[:].rearrange("p (h q) -> p h q", h=2)[:, :, qlo:QT]
                        exb_v = exb[:].rearrange("p (h q) -> p h q", h=2)[:, :, qlo:QT]
                        nc.scalar.activation(exb_v, lg_v, Exp, scale=SCALE,
                                             bias=ebias[:])
                        if fillers:
                            n_pop = math.ceil(len(fillers) / (4 - kb))
                            for _ in range(n_pop):
                                fillers.popleft()()
                        nc.gpsimd.affine_select(
                            out=exb_v, in_=exb_v,
                            compare_op=mybir.AluOpType.is_ge,
                            fill=0.0, base=0,
                            pattern=[[0, 2], [1, nq]],
                            channel_multiplier=-1,
                        )
                        nc.tensor.matmul(
                            pvA[:, qlo:QT], lhsT=vb16[kb][:, 0:65],
                            rhs=exb[:, qlo:QT],
                            start=(kb == 0), stop=(kb == 3),
                            skip_group_check=True,
                        )
                        nc.tensor.matmul(
                            pvB[:, qlo:QT], lhsT=vb16[kb][:, 65:130],
                            rhs=exb[:, QT + qlo : 2 * QT],
                            start=(kb == 0), stop=(kb == 3),
                            skip_group_check=True,
                        )
                    n_kb = 0  # skip the fp8 loop below

                ex8 = None
                pending_pvs = deque()
                for kb in range(n_kb):
                    kvs = slice((kb % 4) * KB, (kb % 4 + 1) * KB)
                    ktile = kts[kb // 4]
                    r = kb % 2
                    off = kb - 4 * qi
                    qlo = max(off - r, 0) * KB  # pair-aligned trim
                    nq = QT - qlo
                    lg = lgp.tile([128, 2 * QT], f32, name="lg", tag="lg")
                    nc.tensor.matmul(
                        lg[:, qlo:QT], lhsT=ktile[0:64, kvs],
                        rhs=qts[qi][0:64, qlo:QT],
                        start=True, stop=True,
                    )
                    nc.tensor.matmul(
                        lg[:, QT + qlo : 2 * QT], lhsT=ktile[64:128, kvs],
                        rhs=qts[qi][64:128, qlo:QT],
                        start=True, stop=True,
                    )
                    if r == 0:
                        ex8 = expp.tile([128, 4 * QT], fp8, name="ex8", tag="exp")
                    exv = ex8[:].rearrange("p (h t q) -> p h t q", h=2, t=2)
                    lg_v = lg[:].rearrange("p (h q) -> p h q", h=2)[:, :, qlo:QT]
                    ex_v = exv[:, :, r, qlo:QT]
                    eng = pick_engine()
                    if eng == "act":
                        nc.scalar.activation(ex_v, lg_v, Exp, scale=math.log(2) / 8.0,
                                             bias=ebias[:])
                    else:
                        ex_u8 = ex8[:].bitcast(u8).rearrange(
                            "p (h t q) -> p h t q", h=2, t=2
                        )[:, :, r, qlo:QT]
                        nc.vector.tensor_scalar(
                            ex_u8, lg_v, B_AFF, 0.0, Alu.add, Alu.max
                        )

                    # emit PV lagging TWO pairs behind its exp, so the
                    # in-order PE queue never stalls on the exp/mask lanes
                    if r == 1 and len(pending_pvs) >= 2:
                        pending_pvs.popleft()()

                    if fillers:
                        n_pop = math.ceil(len(fillers) / (n_kb - kb))
                        for _ in range(n_pop):
                            fillers.popleft()()

                    if off >= 0:
                        nc.gpsimd.affine_select(
                            out=ex_v,
                            in_=ex_v,
                            compare_op=mybir.AluOpType.is_ge,
                            fill=0.0,
                            base=qlo - off * KB,
                            pattern=[[0, 2], [1, nq]],
                            channel_multiplier=-1,
                        )
                    if r == 1:
                        def make_pv(kp=kb // 2, qlo=qlo, exv=exv, last=(kb == n_kb - 1)):
                            def f():
                                vv = v8s[kp][:].rearrange("p (t d) -> p t d", t=2)
                                nc.tensor.matmul(
                                    pvA[:, qlo:QT], lhsT=vv[:, :, 0:65],
                                    rhs=exv[:, 0, :, qlo:QT],
                                    start=(kp == 0), stop=last,
                                    perf_mode=DoubleRow,
                                    skip_group_check=True,
                                )
                                nc.tensor.matmul(
                                    pvB[:, qlo:QT], lhsT=vv[:, :, 80:145],
                                    rhs=exv[:, 1, :, qlo:QT],
                                    start=(kp == 0), stop=last,
                                    perf_mode=DoubleRow,
                                    skip_group_check=True,
                                )

                            return f

                        pending_pvs.append(make_pv())
                # the final PV pair (diag, qlo=256) only writes cols 256:,
                # so the first norm half starts while it still runs
                if n_kb > 0:
                    while len(pending_pvs) > 1:
                        pending_pvs.popleft()()
                    tl = qi == NQ - 1
                    norm_emit(pvA, pvB, qi, 0, QT // 2, tail=tl)
                    pending_pvs.popleft()()
                    while fillers:
                        fillers.popleft()()
                    if tl:
                        for f in oproj_units(qi, sbis=(0, 1), tail=True):
                            f()
                    norm_emit(pvA, pvB, qi, QT // 2, QT, tail=tl)
                else:
                    while fillers:
                        fillers.popleft()()
                    norm_emit(pvA, pvB, qi, 0, QT)

            # epilogue: o_proj of the final tile's second half
            for f in oproj_units(NQ - 1, sbis=(2, 3), tail=True):
                f()

    nc.compile()
    return nc


def _host_inputs(x, Wq, Wk, Wv, Wo):
    x2 = np.asarray(x, dtype=np.float32).reshape(S, E)
    xT = np.ascontiguousarray(x2.T)  # [E, S] fp32
    # chunk-major [128, 8*S]
    xT_cm = xT.reshape(8, 128, S).transpose(1, 0, 2).reshape(128, 8 * S)
    xT8 = np.ascontiguousarray(xT_cm).astype(F8E4)
    xTb = np.ascontiguousarray(
        xT.reshape(8, 128, S)[:, :, 0:QT].transpose(1, 0, 2).reshape(128, 8 * QT)
    ).astype(BF16)

    def pack(wT):  # [1024(e), 128(d)] -> [128(p), ec*128+d]
        return np.ascontiguousarray(
            wT.reshape(8, 128, 128).transpose(1, 0, 2).reshape(128, 1024)
        )

    in_maps = []
    for c in range(NCORES):
        r = slice(128 * c, 128 * (c + 1))
        wq_t = np.asarray(Wq, np.float32)[r, :].T
        wk_t = np.asarray(Wk, np.float32)[r, :].T
        wv_t = np.asarray(Wv, np.float32)[r, :].T
        in_maps.append(
            {
                "xT8": xT8,
                "xTb": xTb,
                "wq8": pack(WSC * wq_t).astype(F8E4),
                "wk8": pack(WSC * wk_t).astype(F8E4),
                "wv8": pack(WSC * wv_t).astype(F8E4),
                "wqb": pack(wq_t).astype(BF16),
                "wkb": pack(wk_t).astype(BF16),
                "wvb": pack(wv_t).astype(BF16),
                "wo": np.ascontiguousarray(
                    np.asarray(Wo, np.float32)[:, r].T
                ).astype(BF16),
            }
        )
    return in_maps


def _get_nc():
    if "nc" not in _CACHE:
        _CACHE["nc"] = _build_nc()
    return _CACHE["nc"]


def run(x, Wq, Wk, Wv, Wo, trace=False, trace_kwargs=None):
    """Build+run the SPMD kernel; returns (full_output [S,E] f32, BassKernelResults)."""
    from concourse.bass_utils import run_bass_kernel_spmd

    nc = _get_nc()
    in_maps = _host_inputs(x, Wq, Wk, Wv, Wo)
    res = run_bass_kernel_spmd(
        nc,
        in_maps,
        list(range(NCORES)),
        trace=trace,
        **(trace_kwargs or {}),
    )
    out = np.zeros((S, E), dtype=np.float32)
    for c in range(NCORES):
        out += res.results[c]["out"].astype(np.float32)
    return out, res


def kernel(x, Wq, Wk, Wv, Wo):
    out, _ = run(x, Wq, Wk, Wv, Wo)
    return out.reshape(1, S, E).astype(np.float32)


# revision 47
# speedup vs baseline: 1.0411x; 1.0411x over previous
"""Causal self-attention Trainium2 kernel (B=1, S=4096, E=1024, H=16, D=64).

Sharding: tensor-parallel over heads - 2 heads per core (8 cores).
Each core computes Q/K/V for its 2 heads, causal attention, and a partial
o_proj over its 128 output-feature slice; the host sums the 8 partials.

Key structure (per core), v2:
  * Q/K/V projections run as fp8e4 DoubleRow matmuls (x and 64*W shipped in
    fp8; contraction 256 per matmul, 0.5 cyc/col) -> bf16 K/Q tiles carry a
    x64 scale that is folded into the exp scale (logits x4096); V is scaled
    back by 1/64 in its PSUM->fp8 cast.
  * q-tile 0 (rows 0-511, low-context -> accuracy-critical) instead uses a
    bf16-exact path: bf16 x / bf16 W projections, exact exp, bf16 V.
  * softmax exp+fp8-quantize is spread across THREE engines: ACT does exact
    exp->fp8e4 for its share; DVE and GPSIMD exploit that fp8e4's bit
    pattern is log-linear, so bits(fp8(e^(s*lg+b))) ~= clamp(a*lg+b', 0, *):
    a 2-instr affine (tensor_scalar mult/add -> bf16, then max(.,0) -> uint8
    written through a bitcast view of the fp8 tile). A uniform bit offset is
    a uniform weight scale and cancels in the softmax normalization.
  * logits computed transposed, both heads packed into PE row-groups
    (rows 0-63 / 64-127) writing separate PSUM banks.
  * causal masking via GPSIMD affine_select on diagonal-band blocks.
  * PV uses fp8e4 DoubleRow matmuls (kv-pair contraction 256) with an
    appended ones column accumulating the softmax denominator in row 64.
  * normalize via reciprocal_approx_fast + stream_shuffle broadcast.
  * per-q-tile projections and o_proj matmuls interleave as PE fillers.
"""

import math
import sys
from collections import deque

import numpy as np

for _p in ("/opt/trn_rl_repo", "/opt/trn_rl_repo/concourse"):
    if _p not in sys.path:
        sys.path.insert(0, _p)

import ml_dtypes

BF16 = ml_dtypes.bfloat16
F8E4 = ml_dtypes.float8_e4m3

S = 4096
E = 1024
H = 16
D = 64
NCORES = 8
QT = 512  # query tile (free dim of logits matmuls)
NQ = S // QT  # 8
KB = 128  # kv block (partition dim of logits tiles)
SCALE = 1.0 / math.sqrt(D)
LOG2E = math.log2(math.e)

EXPB = -1.5  # exp(scale*logit + EXPB); uniform factor cancels in softmax
B_AFF = 8.0 * (7.0 + LOG2E * EXPB)  # fp8e4 bits bias (38.688)
A_RAW = 8.0 * LOG2E * SCALE  # bits per raw-logit unit (1.4427)
A_FP8 = A_RAW / 4096.0  # fp8-proj logits carry x4096
WSC = 64.0  # host weight pre-scale for fp8

# K/Q casts pre-scale by M_KQ so the fp8-path logits arrive in fp8e4 "bits"
# scale: bits = LG_scaled + B_AFF. The DVE exp is then ONE tensor_scalar
# (add B, max 0) -> uint8; ACT uses exact exp (scale ln2/8 recovers nats).
M_KQ = math.sqrt(A_RAW) / WSC
# exp engine shares for non-diagonal tiles (diag + qi0 go to ACT).
W_ACT, W_DVE = 0.55, 0.45

_CACHE = {}


def _build_nc():
    import concourse.tile as tile
    from concourse import bacc, mybir

    dt = mybir.dt
    f32 = dt.float32
    bf16 = dt.bfloat16
    fp8 = dt.float8e4
    u8 = dt.uint8
    Exp = mybir.ActivationFunctionType.Exp
    Copy = mybir.ActivationFunctionType.Copy
    DoubleRow = mybir.MatmulPerfMode.DoubleRow
    Alu = mybir.AluOpType

    nc = bacc.Bacc("TRN2", target_bir_lowering=False, debug=False, num_devices=NCORES)

    xT8_d = nc.dram_tensor("xT8", [128, 8 * S], fp8, kind="ExternalInput")
    xTb_d = nc.dram_tensor("xTb", [128, 8 * QT], bf16, kind="ExternalInput")
    wq8_d = nc.dram_tensor("wq8", [128, 1024], fp8, kind="ExternalInput")
    wk8_d = nc.dram_tensor("wk8", [128, 1024], fp8, kind="ExternalInput")
    wv8_d = nc.dram_tensor("wv8", [128, 1024], fp8, kind="ExternalInput")
    wqb_d = nc.dram_tensor("wqb", [128, 1024], bf16, kind="ExternalInput")
    wkb_d = nc.dram_tensor("wkb", [128, 1024], bf16, kind="ExternalInput")
    wvb_d = nc.dram_tensor("wvb", [128, 1024], bf16, kind="ExternalInput")
    wo_d = nc.dram_tensor("wo", [128, 1024], bf16, kind="ExternalInput")
    out_d = nc.dram_tensor("out", [S, E], bf16, kind="ExternalOutput")

    with tile.TileContext(nc) as tc:
        from contextlib import ExitStack

        with ExitStack() as ctx:
            sb = ctx.enter_context(tc.tile_pool(name="sb", bufs=1))
            lgp = ctx.enter_context(tc.tile_pool(name="lgp", bufs=2, space="PSUM"))
            ps = ctx.enter_context(tc.tile_pool(name="ps", bufs=2, space="PSUM"))
            pvp = ctx.enter_context(tc.tile_pool(name="pvp", bufs=2, space="PSUM"))
            expp = ctx.enter_context(tc.tile_pool(name="expp", bufs=8))
            tmpp = ctx.enter_context(tc.tile_pool(name="tmpp", bufs=4))
            normp = ctx.enter_context(tc.tile_pool(name="normp", bufs=3))
            ostp = ctx.enter_context(tc.tile_pool(name="ostp", bufs=4))

            # ---- persistent SBUF tensors + input DMA ----
            wq8_sb = sb.tile([128, 1024], fp8, name="wq8_sb", tag="wq8_sb")
            wk8_sb = sb.tile([128, 1024], fp8, name="wk8_sb", tag="wk8_sb")
            wv8_sb = sb.tile([128, 1024], fp8, name="wv8_sb", tag="wv8_sb")
            wqb_sb = sb.tile([128, 1024], bf16, name="wqb_sb", tag="wqb_sb")
            wkb_sb = sb.tile([128, 1024], bf16, name="wkb_sb", tag="wkb_sb")
            wvb_sb = sb.tile([128, 1024], bf16, name="wvb_sb", tag="wvb_sb")
            wo_sb = sb.tile([128, 1024], bf16, name="wo_sb", tag="wo_sb")
            nc.sync.dma_start(wkb_sb[:], wkb_d[:])
            nc.sync.dma_start(wqb_sb[:], wqb_d[:])
            nc.sync.dma_start(wvb_sb[:], wvb_d[:])
            nc.sync.dma_start(wk8_sb[:], wk8_d[:])
            nc.sync.dma_start(wq8_sb[:], wq8_d[:])
            nc.sync.dma_start(wv8_sb[:], wv8_d[:])
            nc.sync.dma_start(wo_sb[:], wo_d[:])

            xtb = sb.tile([128, 8 * QT], bf16, name="xtb", tag="xtb")
            nc.sync.dma_start(xtb[:], xTb_d[:])
            xt8 = sb.tile([128, 8 * S], fp8, name="xt8", tag="xt8")
            for ec in range(8):
                nc.sync.dma_start(
                    xt8[:, ec * S : ec * S + 2 * QT],
                    xT8_d[:, ec * S : ec * S + 2 * QT],
                )
            for ec in range(8):
                nc.sync.dma_start(
                    xt8[:, ec * S + 2 * QT : (ec + 1) * S],
                    xT8_d[:, ec * S + 2 * QT : (ec + 1) * S],
                )
            x8v = xt8[:].rearrange("p (c s) -> p c s", c=8)
            xbv = xtb[:].rearrange("p (c s) -> p c s", c=8)

            kts = [sb.tile([128, QT], bf16, name=f"kt{i}", tag=f"kt{i}") for i in range(NQ)]
            qts = [sb.tile([128, QT], bf16, name=f"qt{i}", tag=f"qt{i}") for i in range(NQ)]
            kt0x = sb.tile([128, QT], bf16, name="kt0x", tag="kt0x")
            qt0x = sb.tile([128, QT], bf16, name="qt0x", tag="qt0x")
            aots = [sb.tile([128, QT], bf16, name=f"ao{i}", tag=f"ao{i}") for i in range(NQ)]
            # V for DoubleRow PV: one fp8 tile per kv-block PAIR, layout
            # [128(s within block), pair-slot(2) x 160]: head A V at d 0-63 +
            # ones col 64; head B V at 80-143 + ones col 144.
            v8s = []
            for i in range(16):
                v = sb.tile([128, 320], fp8, name=f"v{i}", tag=f"v{i}")
                vv = v[:].rearrange("p (t d) -> p t d", t=2)
                nc.vector.memset(vv[:, :, 64:65], 1.0)
                nc.vector.memset(vv[:, :, 144:145], 1.0)
                v8s.append(v)
            # bf16 V for q-tile 0 (exact path)
            vb16 = []
            for i in range(4):
                v = sb.tile([128, 130], bf16, name=f"vb{i}", tag=f"vb{i}")
                nc.vector.memset(v[:, 64:65], 1.0)
                nc.vector.memset(v[:, 129:130], 1.0)
                vb16.append(v)

            bcseed = sb.tile([64, QT], f32, name="bcseed", tag="bcseed")
            nc.vector.memset(bcseed[:], 0.0)
            ebias = sb.tile([128, 1], f32, name="ebias", tag="ebias")
            nc.vector.memset(ebias[:], EXPB)

            # ---- filler-unit constructors ----
            def kq8_units(dst, w8, st):
                # fp8 DoubleRow K/Q projection of q-tile st -> dst bf16 (x64)
                cols = slice(st * QT, (st + 1) * QT)
                w8v = w8[:].rearrange("p (c d) -> p c d", c=8)
                state = {}

                def mm(i):
                    def f():
                        if i == 0:
                            state["t"] = ps.tile([128, QT], f32, name="ps_kq", tag="ps")
                        nc.tensor.matmul(
                            state["t"][:],
                            lhsT=w8v[:, 2 * i : 2 * i + 2, :],
                            rhs=x8v[:, 2 * i : 2 * i + 2, cols],
                            start=(i == 0),
                            stop=(i == 3),
                            perf_mode=DoubleRow,
                        )

                    return f

                def cast():
                    nc.scalar.activation(
                        dst[:], state["t"][:], Copy, scale=M_KQ
                    )

                return [mm(i) for i in range(4)] + [cast]

            def v8_units(kb):
                # fp8 DoubleRow V projection of kv block kb -> v8s (x1/64)
                scols = slice(kb * 128, (kb + 1) * 128)
                w8v = wv8_sb[:].rearrange("p (c d) -> p c d", c=8)
                state = {}

                def mm(i):
                    def f():
                        if i == 0:
                            state["t"] = ps.tile([128, 128], f32, name="ps_v", tag="ps")
                        nc.tensor.matmul(
                            state["t"][:],
                            lhsT=x8v[:, 2 * i : 2 * i + 2, scols],
                            rhs=w8v[:, 2 * i : 2 * i + 2, :],
                            start=(i == 0),
                            stop=(i == 3),
                            perf_mode=DoubleRow,
                        )

                    return f

                def cast():
                    # both head slices in one strided instr: d 0:64 -> slot
                    # cols 0:64, d 64:128 -> cols 80:144 (stride-80 pair dim)
                    r = kb % 2
                    src = state["t"][:].rearrange("p (h d) -> p h d", h=2)
                    dst = v8s[kb // 2][:, r * 160 : (r + 1) * 160].rearrange(
                        "p (h d) -> p h d", h=2
                    )[:, :, 0:64]
                    nc.scalar.activation(dst, src, Copy, scale=1.0 / WSC)

                return [mm(i) for i in range(4)] + [cast]

            def kqx_units(dst, wb):
                # bf16-exact K/Q projection of q-tile 0
                state = {}

                def mm(ec):
                    def f():
                        if ec == 0:
                            state["t"] = ps.tile([128, QT], f32, name="ps_kqx", tag="ps")
                        nc.tensor.matmul(
                            state["t"][:],
                            lhsT=wb[:, ec * 128 : (ec + 1) * 128],
                            rhs=xbv[:, ec, :],
                            start=(ec == 0),
                            stop=(ec == 7),
                        )

                    return f

                def cast():
                    nc.vector.tensor_copy(dst[:], state["t"][:])

                return [mm(ec) for ec in range(8)] + [cast]

            def vx_units(kb):
                # bf16-exact V block kb (0..3) -> vb16
                state = {}

                def mm(ec):
                    def f():
                        if ec == 0:
                            state["t"] = ps.tile([128, 128], f32, name="ps_vx", tag="ps")
                        nc.tensor.matmul(
                            state["t"][:],
                            lhsT=xbv[:, ec, kb * 128 : (kb + 1) * 128],
                            rhs=wvb_sb[:, ec * 128 : (ec + 1) * 128],
                            start=(ec == 0),
                            stop=(ec == 7),
                        )

                    return f

                def cast():
                    nc.vector.tensor_copy(vb16[kb][:, 0:64], state["t"][:, 0:64])
                    nc.vector.tensor_copy(vb16[kb][:, 65:129], state["t"][:, 64:128])

                return [mm(ec) for ec in range(8)] + [cast]

            def oproj_units(qj, sbis=range(4), tail=False):
                units = []
                for sbi in sbis:
                    for half in range(2):

                        def f(sbi=sbi, half=half):
                            srow = qj * QT + sbi * 128
                            po = ps.tile([128, 512], f32, name="po", tag="ps")
                            nc.tensor.matmul(
                                po[:],
                                lhsT=aots[qj][:, sbi * 128 : (sbi + 1) * 128],
                                rhs=wo_sb[:, half * 512 : (half + 1) * 512],
                                start=True,
                                stop=True,
                            )
                            ost = ostp.tile([128, 512], bf16, name="ost", tag="ost")
                            if (tail and half == 0) or (sbi + half) % 2 == 0:
                                nc.scalar.copy(ost[:], po[:])
                            else:
                                nc.vector.tensor_copy(ost[:], po[:])
                            nc.sync.dma_start(
                                out_d[srow : srow + 128, half * 512 : (half + 1) * 512],
                                ost[:],
                            )

                        units.append(f)
                return units

            def proj8_units(qi2):
                u = []
                u += kq8_units(kts[qi2], wk8_sb, qi2)
                u += kq8_units(qts[qi2], wq8_sb, qi2)
                for kb in range(4 * qi2, 4 * qi2 + 4):
                    u += v8_units(kb)
                return u

            def norm_emit(pvA, pvB, qj, c0, c1, tail=False):
                for pv, r0 in ((pvA, 0), (pvB, 64)):
                    den_sb = normp.tile([1, QT], f32, name="den_sb", tag="den")
                    if tail:
                        nc.scalar.copy(den_sb[:, c0:c1], pv[64:65, c0:c1])
                    else:
                        nc.vector.tensor_copy(den_sb[:, c0:c1], pv[64:65, c0:c1])
                    nc.vector.reciprocal_approx_fast(
                        bcseed[0:1, c0:c1], den_sb[:, c0:c1]
                    )
                    nc.vector.tensor_copy(bcseed[32:33, c0:c1], bcseed[0:1, c0:c1])
                    bcast = normp.tile([64, QT], f32, name="bcast", tag="bcast")
                    nc.vector.stream_shuffle(
                        bcast[:, c0:c1], bcseed[:, c0:c1], [0] * 32
                    )
                    nc.vector.tensor_mul(
                        aots[qj][r0 : r0 + 64, c0:c1], pv[0:64, c0:c1],
                        bcast[:, c0:c1],
                    )

            # weighted round-robin chooser for non-diag exp engine
            eng_credit = {"act": 0.0, "dve": 0.0}
            eng_w = {"act": W_ACT, "dve": W_DVE}

            def pick_engine():
                for k in eng_credit:
                    eng_credit[k] += eng_w[k]
                best = max(eng_credit, key=lambda k: eng_credit[k])
                eng_credit[best] -= 1.0
                return best

            # ---- prologue: q-tile 0 exact projections + fp8 tile-0 ----
            for f in kqx_units(kt0x, wkb_sb):
                f()
            for f in kqx_units(qt0x, wqb_sb):
                f()
            for kb in range(4):
                for f in vx_units(kb):
                    f()
            for f in proj8_units(0):
                f()

            # ---- main loop over q-tiles ----
            for qi in range(NQ):
                fillers = deque()
                if qi + 1 < NQ:
                    fillers.extend(proj8_units(qi + 1))
                if qi >= 1:
                    fillers.extend(oproj_units(qi - 1))

                n_kb = 4 * (qi + 1)
                pvA = pvp.tile([65, QT], f32, name="pvA", tag="pv")
                pvB = pvp.tile([65, QT], f32, name="pvB", tag="pv")
                if qi == 0:
                    # bf16-exact path for rows 0-511
                    for kb in range(4):
                        kvs = slice(kb * KB, (kb + 1) * KB)
                        off = kb
                        qlo = off * KB
                        nq = QT - qlo
                        lg = lgp.tile([128, 2 * QT], f32, name="lg", tag="lg")
                        nc.tensor.matmul(
                            lg[:, qlo:QT], lhsT=kt0x[0:64, kvs],
                            rhs=qt0x[0:64, qlo:QT], start=True, stop=True,
                        )
                        nc.tensor.matmul(
                            lg[:, QT + qlo : 2 * QT], lhsT=kt0x[64:128, kvs],
                            rhs=qt0x[64:128, qlo:QT], start=True, stop=True,
                        )
                        exb = expp.tile([128, 2 * QT], bf16, name="exb", tag="exp")
                        lg_v = lg[:].rearrange("p (h q) -> p h q", h=2)[:, :, qlo:QT]
                        exb_v = exb[:].rearrange("p (h q) -> p h q", h=2)[:, :, qlo:QT]
                        nc.scalar.activation(exb_v, lg_v, Exp, scale=SCALE,
                                             bias=ebias[:])
                        if fillers:
                            n_pop = math.ceil(len(fillers) / (4 - kb))
                            for _ in range(n_pop):
                                fillers.popleft()()
                        nc.gpsimd.affine_select(
                            out=exb_v, in_=exb_v,
                            compare_op=mybir.AluOpType.is_ge,
                            fill=0.0, base=0,
                            pattern=[[0, 2], [1, nq]],
                            channel_multiplier=-1,
                        )
                        nc.tensor.matmul(
                            pvA[:, qlo:QT], lhsT=vb16[kb][:, 0:65],
                            rhs=exb[:, qlo:QT],
                            start=(kb == 0), stop=(kb == 3),
                            skip_group_check=True,
                        )
                        nc.tensor.matmul(
                            pvB[:, qlo:QT], lhsT=vb16[kb][:, 65:130],
                            rhs=exb[:, QT + qlo : 2 * QT],
                            start=(kb == 0), stop=(kb == 3),
                            skip_group_check=True,
                        )
                    n_kb = 0  # skip the fp8 loop below

                ex8 = None
                pending_pvs = deque()
                for kb in range(n_kb):
                    kvs = slice((kb % 4) * KB, (kb % 4 + 1) * KB)
                    ktile = kts[kb // 4]
                    r = kb % 2
                    off = kb - 4 * qi
                    qlo = max(off - r, 0) * KB  # pair-aligned trim
                    nq = QT - qlo
                    lg = lgp.tile([128, 2 * QT], f32, name="lg", tag="lg")
                    nc.tensor.matmul(
                        lg[:, qlo:QT], lhsT=ktile[0:64, kvs],
                        rhs=qts[qi][0:64, qlo:QT],
                        start=True, stop=True,
                    )
                    nc.tensor.matmul(
                        lg[:, QT + qlo : 2 * QT], lhsT=ktile[64:128, kvs],
                        rhs=qts[qi][64:128, qlo:QT],
                        start=True, stop=True,
                    )
                    if r == 0:
                        ex8 = expp.tile([128, 4 * QT], fp8, name="ex8", tag="exp")
                    exv = ex8[:].rearrange("p (h t q) -> p h t q", h=2, t=2)
                    lg_v = lg[:].rearrange("p (h q) -> p h q", h=2)[:, :, qlo:QT]
                    ex_v = exv[:, :, r, qlo:QT]
                    eng = pick_engine()
                    if eng == "act":
                        nc.scalar.activation(ex_v, lg_v, Exp, scale=math.log(2) / 8.0,
                                             bias=ebias[:])
                    else:
                        ex_u8 = ex8[:].bitcast(u8).rearrange(
                            "p (h t q) -> p h t q", h=2, t=2
                        )[:, :, r, qlo:QT]
                        nc.vector.tensor_scalar(
                            ex_u8, lg_v, B_AFF, 0.0, Alu.add, Alu.max
                        )

                    # emit PV lagging TWO pairs behind its exp, so the
                    # in-order PE queue never stalls on the exp/mask lanes
                    if r == 1 and len(pending_pvs) >= 2:
                        pending_pvs.popleft()()

                    if fillers:
                        n_pop = math.ceil(len(fillers) / (n_kb - kb))
                        for _ in range(n_pop):
                            fillers.popleft()()

                    if off >= 0:
                        nc.gpsimd.affine_select(
                            out=ex_v,
                            in_=ex_v,
                            compare_op=mybir.AluOpType.is_ge,
                            fill=0.0,
                            base=qlo - off * KB,
                            pattern=[[0, 2], [1, nq]],
                            channel_multiplier=-1,
                        )
                    if r == 1:
                        def make_pv(kp=kb // 2, qlo=qlo, exv=exv, last=(kb == n_kb - 1)):
                            def f():
                                vv = v8s[kp][:].rearrange("p (t d) -> p t d", t=2)
                                nc.tensor.matmul(
                                    pvA[:, qlo:QT], lhsT=vv[:, :, 0:65],
                                    rhs=exv[:, 0, :, qlo:QT],
                                    start=(kp == 0), stop=last,
                                    perf_mode=DoubleRow,
                                    skip_group_check=True,
                                )
                                nc.tensor.matmul(
                                    pvB[:, qlo:QT], lhsT=vv[:, :, 80:145],
                                    rhs=exv[:, 1, :, qlo:QT],
                                    start=(kp == 0), stop=last,
                                    perf_mode=DoubleRow,
                                    skip_group_check=True,
                                )

                            return f

                        pending_pvs.append(make_pv())
                while pending_pvs:
                    pending_pvs.popleft()()
                while fillers:
                    fillers.popleft()()
                if qi == NQ - 1:
                    norm_emit(pvA, pvB, qi, 0, QT // 2, tail=True)
                    for f in oproj_units(qi, sbis=(0, 1), tail=True):
                        f()
                    norm_emit(pvA, pvB, qi, QT // 2, QT, tail=True)
                else:
                    norm_emit(pvA, pvB, qi, 0, QT)

            # epilogue: o_proj of the final tile's second half
            for f in oproj_units(NQ - 1, sbis=(2, 3), tail=True):
                f()

    nc.compile()
    return nc


def _host_inputs(x, Wq, Wk, Wv, Wo):
    x2 = np.asarray(x, dtype=np.float32).reshape(S, E)
    xT = np.ascontiguousarray(x2.T)  # [E, S] fp32
    # chunk-major [128, 8*S]
    xT_cm = xT.reshape(8, 128, S).transpose(1, 0, 2).reshape(128, 8 * S)
    xT8 = np.ascontiguousarray(xT_cm).astype(F8E4)
    xTb = np.ascontiguousarray(
        xT.reshape(8, 128, S)[:, :, 0:QT].transpose(1, 0, 2).reshape(128, 8 * QT)
    ).astype(BF16)

    def pack(wT):  # [1024(e), 128(d)] -> [128(p), ec*128+d]
        return np.ascontiguousarray(
            wT.reshape(8, 128, 128).transpose(1, 0, 2).reshape(128, 1024)
        )

    in_maps = []
    for c in range(NCORES):
        r = slice(128 * c, 128 * (c + 1))
        wq_t = np.asarray(Wq, np.float32)[r, :].T
        wk_t = np.asarray(Wk, np.float32)[r, :].T
        wv_t = np.asarray(Wv, np.float32)[r, :].T
        in_maps.append(
            {
                "xT8": xT8,
                "xTb": xTb,
                "wq8": pack(WSC * wq_t).astype(F8E4),
                "wk8": pack(WSC * wk_t).astype(F8E4),
                "wv8": pack(WSC * wv_t).astype(F8E4),
                "wqb": pack(wq_t).astype(BF16),
                "wkb": pack(wk_t).astype(BF16),
                "wvb": pack(wv_t).astype(BF16),
                "wo": np.ascontiguousarray(
                    np.asarray(Wo, np.float32)[:, r].T
                ).astype(BF16),
            }
        )
    return in_maps


def _get_nc():
    if "nc" not in _CACHE:
        _CACHE["nc"] = _build_nc()
    return _CACHE["nc"]


def run(x, Wq, Wk, Wv, Wo, trace=False, trace_kwargs=None):
    """Build+run the SPMD kernel; returns (full_output [S,E] f32, BassKernelResults)."""
    from concourse.bass_utils import run_bass_kernel_spmd

    nc = _get_nc()
    in_maps = _host_inputs(x, Wq, Wk, Wv, Wo)
    res = run_bass_kernel_spmd(
        nc,
        in_maps,
        list(range(NCORES)),
        trace=trace,
        **(trace_kwargs or {}),
    )
    out = np.zeros((S, E), dtype=np.float32)
    for c in range(NCORES):
        out += res.results[c]["out"].astype(np.float32)
    return out, res


def kernel(x, Wq, Wk, Wv, Wo):
    out, _ = run(x, Wq, Wk, Wv, Wo)
    return out.reshape(1, S, E).astype(np.float32)
